# revision 39
# baseline (speedup 1.0000x reference)
"""DGT block (dynamic graph transformer) Bass kernel for Trainium2.

Sharding: 8 cores = 4 batches x 2 query-halves. Each core handles one
batch's feats/pos and one half (2048) of the queries.

v3:
  - kNN scores via float32r matmul (1 cyc/row on PE vs fp32's 4).
  - top-16 without MaxIndex full-row scans: the PSUM->SBUF score drain
    writes fp16 keys (score - self_score + 8; top keys in (0,8]) into
    the hi 16-bit lanes of an f32 "packed" tile whose lo lanes hold the
    column index. Positive-float ordering == lexicographic (key, idx),
    so chunked max8 + merge yields values carrying their own indices.
  - idx matrix transposed with the DMA XBAR (dma_start_transpose).
  - softmax sums as strided pairwise-add trees in DVE 2x (16-bit) mode;
    ww adds + idx replication on gpsimd (Pool).
  - topk pipeline runs 2 tiles ahead of the MLP pipeline, and starts
    during setup (it only needs rhs65/lhsT65/iota DMAs).
  - MLP zpair PSUM split into 1024-wide double-buffered halves.
"""

import numpy as np
import ml_dtypes

B, N, K, DP, DM, EPS = 4, 4096, 16, 64, 128, 1e-5
NQ = N // 2            # queries per core
TQ = 128               # queries per tile
NT = NQ // TQ          # tiles per core (16)
PAIR = TQ * K          # pairs per tile (2048)
CH = 256               # top-k scan chunk size
NCH = N // CH          # 16 chunks
ROW = 3 * DM           # gather-table row elems (f16): [gk | v | gp]
KEY_C = 8.0            # key offset: top keys in (0, 8]
AHEAD = 2              # topk pipeline lead (tiles)

_CACHE = {}

bf16 = ml_dtypes.bfloat16


def _fold_bn(p):
    g, be, m, v = p.astype(np.float64)
    s = g / np.sqrt(v + EPS)
    return (s).astype(np.float32), (be - m * s).astype(np.float32)


def _build_bass():
    import concourse.bass as bass
    import concourse.mybir as mybir
    import concourse.bacc as bacc
    from concourse.tile import TileContext

    dt = mybir.dt
    AF = mybir.ActivationFunctionType
    ALU = mybir.AluOpType

    nc = bacc.Bacc("TRN2", target_bir_lowering=False, debug=False, num_devices=8)

    def inp(name, shape, dtype):
        return nc.dram_tensor(name, list(shape), dtype, kind="ExternalInput").ap()

    rhs65_d = inp("rhs65", (DP + 1, N), dt.float32r)
    lhsT65_d = inp("lhsT65", (DP + 1, NQ), dt.float32r)
    feats_own_f32 = inp("feats_own", (DP, NQ), dt.float32)
    fb_own_d = inp("fb_own", (DP, NQ), dt.bfloat16)
    feats_bf = inp("feats_bf", (DP, N), dt.bfloat16)
    pos_bf = inp("pos_bf", (3, N), dt.bfloat16)
    pos_own = inp("pos_own", (3, NQ), dt.bfloat16)
    biasq_d = inp("biasq", (TQ, NT), dt.float32)
    iota_d = inp("iotapk", (TQ, N), dt.float32)
    w1t_d = inp("W1fT", (DP, DM), dt.bfloat16)
    wkvt_d = inp("WgkvT", (DM, 2 * DM), dt.bfloat16)
    wqt_d = inp("Wg1qT", (DM, DM), dt.bfloat16)
    wd1t_d = inp("Wd1fT", (3, DM), dt.bfloat16)
    wd2t_d = inp("Wd2fT", (DM, DM), dt.bfloat16)
    wg1t_d = inp("Wg1fT", (DM, DM), dt.bfloat16)
    wg2t_d = inp("Wg2fT", (DM, DM), dt.bfloat16)
    w2t_d = inp("W2fT", (DM, DP), dt.bfloat16)
    e_d = inp("E", (TQ, PAIR), dt.bfloat16)
    negi_d = inp("negI", (DM, DM), dt.float16)
    b1_d = inp("b1", (DM, 1), dt.float32)
    bd1_d = inp("bd1", (DM, 1), dt.float32)
    bd2_d = inp("bd2", (DM, 1), dt.float32)
    bg1_d = inp("bg1", (DM, 1), dt.float32)
    bg2_d = inp("bg2", (DM, 1), dt.float32)
    b2_d = inp("b2", (DP, 1), dt.float32)

    out_d = nc.dram_tensor("out", [DP, NQ], dt.float32, kind="ExternalOutput").ap()

    f32, f16, bft, i16 = dt.float32, dt.float16, dt.bfloat16, dt.int16

    with TileContext(nc) as tc:
        with (
            tc.tile_pool(name="const", bufs=1) as cpool,
            tc.tile_pool(name="persist", bufs=1) as ppool,
            tc.tile_pool(name="dram", bufs=1, space="DRAM") as dpool,
        ):
            # hot-path DMAs first (queue order == emission order)
            b1 = cpool.tile_from(b1_d)
            w1t = cpool.tile_from(w1t_d)
            biasq = cpool.tile_from(biasq_d)
            wkvt = cpool.tile_from(wkvt_d)
            wqt = cpool.tile_from(wqt_d)
            wd1t = cpool.tile_from(wd1t_d)
            wd2t = cpool.tile_from(wd2t_d)
            wg1t = cpool.tile_from(wg1t_d)
            wg2t = cpool.tile_from(wg2t_d)
            w2t = cpool.tile_from(w2t_d)
            emat = cpool.tile_from(e_d)
            negi = cpool.tile_from(negi_d)
            rhs65 = ppool.tile_from(rhs65_d)
            lhsT65 = ppool.tile_from(lhsT65_d)
            bd1 = cpool.tile_from(bd1_d)
            bd2 = cpool.tile_from(bd2_d)
            bg1 = cpool.tile_from(bg1_d)
            bg2 = cpool.tile_from(bg2_d)
            b2 = cpool.tile_from(b2_d)

            gqT = ppool.tile([TQ, NQ], bft)
            gpT = ppool.tile([TQ, NQ], bft)
            res_all = ppool.tile([DM, NQ], bft)
            table = dpool.tile([N, ROW], f16)

            with (
                tc.tile_pool(name="setupA", bufs=2) as apool,
                tc.tile_pool(name="score", bufs=2) as spool,
                tc.tile_pool(name="xpool", bufs=1) as xpool,
                tc.tile_pool(name="gath", bufs=3) as gpool,
                tc.tile_pool(name="pair", bufs=2) as prpool,
                tc.tile_pool(name="topk", bufs=3) as kpool,
                tc.tile_pool(name="outp", bufs=2) as opool,
                tc.tile_pool(name="ps_s", bufs=2, space="PSUM") as pss,
                tc.tile_pool(name="ps_pair", bufs=2, space="PSUM") as psp,
            ):
                # ---------- topk emitter (needs only rhs65/lhsT65/iota) ----------
                def emit_topk(t):
                    packed = spool.tile([TQ, N], f32, tag="pk")
                    if t < 2:
                        nc.scalar.dma_start(out=packed[:], in_=iota_d)
                    pk16 = packed[:].bitcast(i16).rearrange(
                        "p (w two) -> p w two", two=2)
                    hi = pk16[:, :, 1:2].bitcast(f16).rearrange("p w 1 -> p w")
                    for s in range(4):
                        ps = pss.tile([TQ, 1024], f32, tag="pssc")
                        nc.tensor.matmul(ps[:, 0:512], lhsT65[:, bass.ts(t, TQ)],
                                         rhs65[:, bass.ts(2 * s, 512)],
                                         start=True, stop=True)
                        nc.tensor.matmul(ps[:, 512:1024], lhsT65[:, bass.ts(t, TQ)],
                                         rhs65[:, bass.ts(2 * s + 1, 512)],
                                         start=True, stop=True)
                        if s >= 2:
                            nc.vector.tensor_scalar(
                                out=hi[:, bass.ts(s, 1024)], in0=ps[:],
                                scalar1=biasq[:, t:t + 1], scalar2=None,
                                op0=ALU.add)
                        else:
                            nc.scalar.activation(
                                hi[:, bass.ts(s, 1024)], ps[:], AF.Prelu,
                                bias=biasq[:, t:t + 1], scale=1.0, alpha=1.0)

                    cand = kpool.tile([TQ, NCH * 8], f32, tag="cand")
                    for c in range(NCH):
                        nc.vector.max(out=cand[:, bass.ts(c, 8)],
                                      in_=packed[:, bass.ts(c, CH)])
                    v8a = kpool.tile([TQ, 8], f32, tag="v8a")
                    nc.vector.max(out=v8a[:], in_=cand[:])
                    nc.vector.match_replace(out=cand[:], in_to_replace=v8a[:],
                                            in_values=cand[:], imm_value=-1e30)
                    v8b = kpool.tile([TQ, 8], f32, tag="v8b")
                    nc.vector.max(out=v8b[:], in_=cand[:])

                    idxf = kpool.tile([TQ, DM], i16, tag="idxf")
                    lo_a = v8a[:].bitcast(i16).rearrange(
                        "p (w two) -> p w two", two=2)[:, :, 0]
                    lo_b = v8b[:].bitcast(i16).rearrange(
                        "p (w two) -> p w two", two=2)[:, :, 0]
                    nc.vector.tensor_copy(out=idxf[:, 0:8], in_=lo_a)
                    nc.vector.tensor_copy(out=idxf[:, 8:16], in_=lo_b)
                    nc.gpsimd.tensor_copy(out=idxf[:, 16:32], in_=idxf[:, 0:16])
                    nc.gpsimd.tensor_copy(out=idxf[:, 32:64], in_=idxf[:, 0:32])
                    nc.gpsimd.tensor_copy(out=idxf[:, 64:128], in_=idxf[:, 0:64])
                    idx16 = kpool.tile([TQ, TQ], i16, tag="idx16")
                    nc.sync.dma_start_transpose(out=idx16[:], in_=idxf[:])
                    return idx16

                # ---------- MLP emitter ----------
                def emit_mlp(t, idx16):
                    gkv = []
                    for gh in range(4):
                        gt = gpool.tile([DM, 3, 512], f16, tag=f"gkv{gh}")
                        nc.gpsimd.dma_gather(
                            out_ap=gt[:], in_ap=table[:],
                            idxs_ap=idx16[:, bass.ts(gh, 32)],
                            num_idxs=512, num_idxs_reg=512, elem_size=ROW,
                            transpose=True)
                        gkv.append(gt)

                    h1 = prpool.tile([DM, PAIR], bft, tag="h1")
                    for half in range(2):
                        zp = psp.tile([DM, PAIR // 2], f32, tag="zpair")
                        for hh in (2 * half, 2 * half + 1):
                            sl = bass.ts(hh - 2 * half, 512)
                            nc.tensor.matmul(zp[:, sl], gpT[:, bass.ts(t, TQ)],
                                             emat[:, bass.ts(hh, 512)],
                                             start=True, stop=False)
                            nc.tensor.matmul(zp[:, sl], negi[:],
                                             gkv[hh][:, 2, :], start=False, stop=True)
                        nc.scalar.activation(h1[:, bass.ts(half, 1024)], zp[:],
                                             AF.Prelu, bias=bd1[:], scale=1.0,
                                             alpha=0.2)
                    pe = prpool.tile([DM, PAIR], bft, tag="pe")
                    for half in range(2):
                        zp = psp.tile([DM, PAIR // 2], f32, tag="zpair")
                        for hh in (2 * half, 2 * half + 1):
                            sl = bass.ts(hh - 2 * half, 512)
                            nc.tensor.matmul(zp[:, sl], wd2t[:],
                                             h1[:, bass.ts(hh, 512)],
                                             start=True, stop=True)
                        nc.scalar.activation(pe[:, bass.ts(half, 1024)], zp[:],
                                             AF.Prelu, bias=bd2[:], scale=1.0,
                                             alpha=0.2)
                    a1 = prpool.tile([DM, PAIR], bft, tag="a1")
                    for half in range(2):
                        zp = psp.tile([DM, PAIR // 2], f32, tag="zpair")
                        for hh in (2 * half, 2 * half + 1):
                            sl = bass.ts(hh - 2 * half, 512)
                            nc.tensor.matmul(zp[:, sl], gqT[:, bass.ts(t, TQ)],
                                             emat[:, bass.ts(hh, 512)],
                                             start=True, stop=False)
                            nc.tensor.matmul(zp[:, sl], negi[:],
                                             gkv[hh][:, 0, :], start=False, stop=False)
                            nc.tensor.matmul(zp[:, sl], wg1t[:],
                                             pe[:, bass.ts(hh, 512)],
                                             start=False, stop=True)
                        nc.scalar.activation(a1[:, bass.ts(half, 1024)], zp[:],
                                             AF.Prelu, bias=bg1[:], scale=1.0,
                                             alpha=0.2)
                    a2 = prpool.tile([DM, PAIR], bft, tag="a2")
                    for half in range(2):
                        zp = psp.tile([DM, PAIR // 2], f32, tag="zpair")
                        for hh in (2 * half, 2 * half + 1):
                            sl = bass.ts(hh - 2 * half, 512)
                            nc.tensor.matmul(zp[:, sl], wg2t[:],
                                             a1[:, bass.ts(hh, 512)],
                                             start=True, stop=True)
                        nc.scalar.activation(a2[:, bass.ts(half, 1024)], zp[:],
                                             AF.Prelu, bias=bg2[:], scale=1.0,
                                             alpha=0.2)
                    ee = a2
                    nc.scalar.activation(ee[:], a2[:], AF.Exp, bias=0.0,
                                         scale=1.0 / 64.0)

                    # softmax-normalized weighted sum
                    ee4 = ee[:].rearrange("p (q two k) -> p q two k", two=2, k=8)
                    s8 = kpool.tile([DM, TQ * 8], bft, tag="t8")
                    nc.vector.tensor_add(s8[:].rearrange("p (q k) -> p q k", k=8),
                                         ee4[:, :, 0], ee4[:, :, 1])
                    s83 = s8[:].rearrange("p (q two k) -> p q two k", two=2, k=4)
                    s4 = kpool.tile([DM, TQ * 4], bft, tag="t4")
                    nc.vector.tensor_add(s4[:].rearrange("p (q k) -> p q k", k=4),
                                         s83[:, :, 0], s83[:, :, 1])
                    s43 = s4[:].rearrange("p (q two k) -> p q two k", two=2, k=2)
                    s2 = kpool.tile([DM, TQ * 2], bft, tag="t2")
                    nc.vector.tensor_add(s2[:].rearrange("p (q k) -> p q k", k=2),
                                         s43[:, :, 0], s43[:, :, 1])
                    s23 = s2[:].rearrange("p (q two) -> p q two", two=2)
                    ssum = kpool.tile([DM, TQ], f32, tag="ssum")
                    nc.vector.tensor_add(ssum[:], s23[:, :, 0], s23[:, :, 1])
                    rrec = kpool.tile([DM, TQ], f32, tag="rrec")
                    nc.vector.reciprocal(rrec[:], ssum[:])

                    ww = h1
                    for hh in range(4):
                        nc.gpsimd.tensor_add(ww[:, bass.ts(hh, 512)],
                                             gkv[hh][:, 1, :],
                                             pe[:, bass.ts(hh, 512)])
                    nc.vector.tensor_mul(ww[:], ee[:], ww[:])
                    uu4 = ww[:].rearrange("p (q two k) -> p q two k", two=2, k=8)
                    u8 = kpool.tile([DM, TQ * 8], bft, tag="t8")
                    nc.vector.tensor_add(u8[:].rearrange("p (q k) -> p q k", k=8),
                                         uu4[:, :, 0], uu4[:, :, 1])
                    u83 = u8[:].rearrange("p (q two k) -> p q two k", two=2, k=4)
                    u4 = kpool.tile([DM, TQ * 4], bft, tag="t4")
                    nc.vector.tensor_add(u4[:].rearrange("p (q k) -> p q k", k=4),
                                         u83[:, :, 0], u83[:, :, 1])
                    u43 = u4[:].rearrange("p (q two k) -> p q two k", two=2, k=2)
                    u2 = kpool.tile([DM, TQ * 2], bft, tag="t2")
                    nc.vector.tensor_add(u2[:].rearrange("p (q k) -> p q k", k=2),
                                         u43[:, :, 0], u43[:, :, 1])
                    u23 = u2[:].rearrange("p (q two) -> p q two", two=2)
                    ru = kpool.tile([DM, TQ], f32, tag="ru")
                    nc.vector.tensor_add(ru[:], u23[:, :, 0], u23[:, :, 1])
                    nc.vector.tensor_mul(res_all[:, bass.ts(t, TQ)], ru[:], rrec[:])

                # ---------- topk prefix for tiles 0..AHEAD-1 ----------
                idx16s = {}
                for t in range(AHEAD):
                    idx16s[t] = emit_topk(t)

                # ---------- Phase A setup (overlaps the prefix) ----------
                fbt = xpool.tile([DP, N], bft)
                nc.sync.dma_start(out=fbt[:], in_=feats_bf)
                post = xpool.tile([3, N], bft)
                nc.sync.dma_start(out=post[:], in_=pos_bf)
                poso = xpool.tile([3, NQ], bft)
                nc.sync.dma_start(out=poso[:], in_=pos_own)
                fob = xpool.tile([DP, NQ], bft)
                nc.sync.dma_start(out=fob[:], in_=fb_own_d)

                xfull = xpool.tile([DM, N], bft)
                for s in range(8):
                    ps = psp.tile([DM, PAIR // 2], f32, tag="zpair")
                    nc.tensor.matmul(ps[:, 0:512], w1t[:], fbt[:, bass.ts(s, 512)],
                                     start=True, stop=True)
                    nc.scalar.activation(xfull[:, bass.ts(s, 512)], ps[:, 0:512],
                                         AF.Prelu, bias=b1[:], scale=1.0, alpha=0.2)
                xob = xpool.tile([DM, NQ], bft)
                for s in range(4):
                    ps = psp.tile([DM, PAIR // 2], f32, tag="zpair")
                    nc.tensor.matmul(ps[:, 0:512], w1t[:], fob[:, bass.ts(s, 512)],
                                     start=True, stop=True)
                    nc.scalar.activation(xob[:, bass.ts(s, 512)], ps[:, 0:512],
                                         AF.Prelu, bias=b1[:], scale=1.0, alpha=0.2)

                # gather table: rows [gk | v | gp] in f16
                for c in range(32):
                    tb = psp.tile([DM, PAIR // 2], f32, tag="zpair")
                    pkv = tb[0:TQ, 0:2 * DM]
                    nc.tensor.matmul(pkv, xfull[:, bass.ts(c, TQ)], wkvt[:],
                                     start=True, stop=True)
                    pgp = tb[0:TQ, 2 * DM:ROW]
                    nc.tensor.matmul(pgp, post[:, bass.ts(c, TQ)], wd1t[:],
                                     start=True, stop=True)
                    stg = apool.tile([TQ, ROW], f16, tag="stg")
                    if c % 2 == 0:
                        nc.vector.tensor_copy(out=stg[:, 0:2 * DM], in_=pkv)
                        nc.vector.tensor_copy(out=stg[:, 2 * DM:ROW], in_=pgp)
                    else:
                        nc.scalar.activation(stg[:, 0:2 * DM], pkv, AF.Copy)
                        nc.scalar.activation(stg[:, 2 * DM:ROW], pgp, AF.Copy)
                    nc.sync.dma_start(out=table[bass.ts(c, TQ), :], in_=stg[:])

                # gqT / gpT for own queries
                for c in range(NT):
                    tb = psp.tile([DM, PAIR // 2], f32, tag="zpair")
                    pq = tb[0:TQ, 0:DM]
                    nc.tensor.matmul(pq, xob[:, bass.ts(c, TQ)],
                                     wqt[:], start=True, stop=True)
                    pp = tb[0:TQ, DM:2 * DM]
                    nc.tensor.matmul(pp, poso[:, bass.ts(c, TQ)], wd1t[:],
                                     start=True, stop=True)
                    if c % 2 == 0:
                        nc.vector.tensor_copy(out=gqT[:, bass.ts(c, DM)], in_=pq)
                        nc.vector.tensor_copy(out=gpT[:, bass.ts(c, DM)], in_=pp)
                    else:
                        nc.scalar.activation(gqT[:, bass.ts(c, DM)], pq, AF.Copy)
                        nc.scalar.activation(gpT[:, bass.ts(c, DM)], pp, AF.Copy)

                # ---------- main loop: topk t+AHEAD, mlp t ----------
                for t in range(NT):
                    if t + AHEAD < NT:
                        idx16s[t + AHEAD] = emit_topk(t + AHEAD)
                    emit_mlp(t, idx16s.pop(t))

                # ---------- output (inside main pools; chunked) ----------
                for sch in range(4):
                    ownc = opool.tile([DP, 512], f32, tag="ownc")
                    nc.sync.dma_start(out=ownc[:],
                                      in_=feats_own_f32[:, bass.ts(sch, 512)])
                    po = psp.tile([DM, PAIR // 2], f32, tag="zpair")
                    nc.tensor.matmul(po[0:DP, 0:512], w2t[:],
                                     res_all[:, bass.ts(sch, 512)],
                                     start=True, stop=True)
                    o1c = opool.tile([DP, 512], f32, tag="o1c")
                    nc.scalar.activation(o1c[:], po[0:DP, 0:512], AF.Prelu,
                                         bias=b2[:], scale=1.0, alpha=0.2)
                    nc.vector.tensor_add(o1c[:], o1c[:], ownc[:])
                    nc.sync.dma_start(out=out_d[:, bass.ts(sch, 512)],
                                      in_=o1c[:])


    nc.compile()
    return nc


def _host_prep(inputs):
    s1, b1 = _fold_bn(np.asarray(inputs["bn1"]))
    sd1, bd1 = _fold_bn(np.asarray(inputs["bnd1"]))
    sd2, bd2 = _fold_bn(np.asarray(inputs["bnd2"]))
    sg1, bg1 = _fold_bn(np.asarray(inputs["bng1"]))
    sg2, bg2 = _fold_bn(np.asarray(inputs["bng2"]))
    s2, b2 = _fold_bn(np.asarray(inputs["bn2"]))
    W1f = np.asarray(inputs["W1"]) * s1[:, None]
    Wd1f = np.asarray(inputs["Wd1"]) * sd1[:, None]
    Wd2f = np.asarray(inputs["Wd2"]) * sd2[:, None]
    Wg1f = np.asarray(inputs["Wg1"]) * sg1[:, None]
    Wg2f = np.asarray(inputs["Wg2"]) * sg2[:, None]
    W2f = np.asarray(inputs["W2"]) * s2[:, None]
    Wg1k = (Wg1f @ np.asarray(inputs["Wk"])).astype(np.float32)
    Wg1q = (Wg1f @ np.asarray(inputs["Wq"])).astype(np.float32)
    Wv = np.asarray(inputs["Wv"], np.float32)

    E = np.zeros((TQ, PAIR), np.float32)
    for q in range(TQ):
        E[q, q * K:(q + 1) * K] = 1.0

    iota = np.broadcast_to(
        np.arange(N, dtype=np.uint32), (TQ, N)).astype(np.uint32)

    com = {
        "W1fT": np.ascontiguousarray(W1f.T, dtype=bf16),
        "WgkvT": np.ascontiguousarray(
            np.concatenate([Wg1k.T, Wv.T], axis=1), dtype=bf16),
        "Wg1qT": np.ascontiguousarray(Wg1q.T, dtype=bf16),
        "Wd1fT": np.ascontiguousarray(Wd1f.T, dtype=bf16),
        "Wd2fT": np.ascontiguousarray(Wd2f.T, dtype=bf16),
        "Wg1fT": np.ascontiguousarray(Wg1f.T, dtype=bf16),
        "Wg2fT": np.ascontiguousarray(Wg2f.T, dtype=bf16),
        "W2fT": np.ascontiguousarray(W2f.T, dtype=bf16),
        "E": E.astype(bf16),
        "negI": (-np.eye(DM)).astype(np.float16),
        "iotapk": iota.view(np.float32).copy(),
        "b1": b1.reshape(DM, 1),
        "bd1": bd1.reshape(DM, 1),
        "bd2": bd2.reshape(DM, 1),
        "bg1": bg1.reshape(DM, 1),
        "bg2": bg2.reshape(DM, 1),
        "b2": b2.reshape(DP, 1),
    }

    feats = np.asarray(inputs["feats"], np.float32)
    pos = np.asarray(inputs["pos"], np.float32)
    in_maps = []
    for c in range(8):
        b, h = c // 2, c % 2
        n0 = h * NQ
        fb = feats[b]
        sq = -0.5 * (fb.astype(np.float64) ** 2).sum(axis=0)
        rhs65 = np.empty((DP + 1, N), np.float32)
        rhs65[0:DP] = fb
        rhs65[DP] = sq.astype(np.float32)
        l65 = np.empty((DP + 1, NQ), np.float32)
        l65[0:DP] = fb[:, n0:n0 + NQ]
        l65[DP] = 1.0
        biasq = (KEY_C + sq[n0:n0 + NQ]).astype(np.float32).reshape(NT, TQ).T
        m = dict(com)
        m["rhs65"] = rhs65
        m["lhsT65"] = l65
        m["feats_own"] = np.ascontiguousarray(fb[:, n0:n0 + NQ])
        m["fb_own"] = np.ascontiguousarray(fb[:, n0:n0 + NQ], dtype=bf16)
        m["feats_bf"] = np.ascontiguousarray(fb, dtype=bf16)
        m["pos_bf"] = np.ascontiguousarray(pos[b], dtype=bf16)
        m["pos_own"] = np.ascontiguousarray(pos[b][:, n0:n0 + NQ], dtype=bf16)
        m["biasq"] = np.ascontiguousarray(biasq)
        in_maps.append(m)
    return in_maps


def kernel(**inputs):
    from concourse.bass_utils import run_bass_kernel_spmd

    if "nc" not in _CACHE:
        _CACHE["nc"] = _build_bass()
    nc = _CACHE["nc"]
    in_maps = _host_prep(inputs)
    r = run_bass_kernel_spmd(nc, in_maps, core_ids=list(range(8)),
                             **_CACHE.get("run_kwargs", {}))
    _CACHE["last_result"] = r
    out = np.empty((B, DP, N), np.float32)
    for c in range(8):
        b, h = c // 2, c % 2
        out[b][:, h * NQ:(h + 1) * NQ] = r.results[c]["out"]
    return out
